# revision 1
# baseline (speedup 1.0000x reference)
"""DynamicHead (cosine-attention memory addressing) Trainium2 kernel.

Full-input contract: kernel(**inputs) takes the unsharded inputs and returns
the full [8192, 4, 128] softmax read-weights tensor. Internally the batch dim
is sharded across 8 NeuronCores (pure data parallel); the two small linear
weights are replicated (pre-cast to fp16 on host, standard launch-time prep).

Per-core algorithm (1024 batches):
  hiddenT = PE-transpose(hidden)                       (fp32 -> fp16 on copy)
  keyT    = tanh(W_key^T @ hiddenT + b_key)            -> kT'' zero-padded
            layout [w(2 halves on partitions), (pair, g, h)] fp16
  g       = softplus(W_beta^T hiddenT + b_beta) / sqrt(sum key^2 + eps)
            folded into kT'' (scales the key columns)
  memT    = PE-transpose(mem pairs)                    (fp32 -> fp16 on copy)
  numT    = memT^T @ kT''-slice  (one [128,128]x[128,8] matmul per pair;
            row-halves carry two independent batches, zero-padding in kT''
            keeps them separate) -> PSUM [m, (pair, g, h)]
  v2      = segmented reduce of mem^2 (ACT Square + DVE reduce)
  logits  = numT * rsqrt(v2)  (g already folded in); wc = exp / sum_m exp
            (sum over m via ones-matmul; logits bounded, no max-subtract)
  out     = PE-transpose back to [b, h, m] and DMA out.
"""

import sys

sys.path.insert(0, "/opt/trn_rl_repo")

import numpy as np

import concourse.bacc as bacc
import concourse.mybir as mybir
from concourse import bass_utils
from concourse.tile import TileContext

F32 = mybir.dt.float32
F32R = mybir.dt.float32r
F16 = mybir.dt.float16
AF = mybir.ActivationFunctionType
ALU = mybir.AluOpType
AX = mybir.AxisListType

NCORES = 8
B_FULL = 8192
D = 512
H = 4
M = 128
W = 64
EPS = 1e-6

NB = B_FULL // NCORES  # 1024 batches per core
HALF = NB // 2  # 512
GROUP = 64  # pairs per group
NGRP = HALF // GROUP  # 8
NBT = NB // 128  # 8 b-tiles
NKT = D // 128  # 4 k-tiles


def build_bass(nb=NB):
    import os

    stage = os.environ.get("BASS_STAGE", "full")
    _ranks = {"a": 0, "b": 1, "c1": 2, "c2": 3, "c3": 4, "full": 5}
    srank = _ranks[stage]
    half = nb // 2
    ngrp = half // GROUP
    nbt = nb // 128

    nc = bacc.Bacc("TRN2", target_bir_lowering=False, debug=False, num_devices=NCORES)

    hid = nc.dram_tensor("hidden", [nb, D], F32, kind="ExternalInput")
    mem = nc.dram_tensor("mem", [nb, M, W], F32, kind="ExternalInput")
    wk16d = nc.dram_tensor("wk16", [D, H * W], F16, kind="ExternalInput")
    wb16d = nc.dram_tensor("wb16", [D, H], F16, kind="ExternalInput")
    bkeyd = nc.dram_tensor("bkey", [128, H], F32, kind="ExternalInput")
    bbetad = nc.dram_tensor("bbeta", [128, H], F32, kind="ExternalInput")
    identd = nc.dram_tensor("ident", [128, 128], F32, kind="ExternalInput")
    onescd = nc.dram_tensor("onesc", [128, 1], F16, kind="ExternalInput")
    onesrd = nc.dram_tensor("onesr", [128, 128], F16, kind="ExternalInput")
    bk2wd = nc.dram_tensor("bk2w", [1, 256], F16, kind="ExternalInput")
    bb16d = nc.dram_tensor("bb16", [1, H], F16, kind="ExternalInput")
    wc = nc.dram_tensor("wc", [nb, H, M], F32, kind="ExternalOutput")

    with TileContext(nc) as tc:
        with (
            tc.tile_pool(name="const", bufs=1) as cpool,
            tc.tile_pool(name="pa", bufs=2) as pa,
            tc.tile_pool(name="pb", bufs=2) as pb,
            tc.tile_pool(name="pc", bufs=1) as pc,
            tc.tile_pool(name="pp", bufs=1, space="PSUM") as pp,
        ):
            # ---- constants ----
            ident = cpool.tile([128, 128], F32, name="ident")
            nc.sync.dma_start(ident[:], identd[:, :])
            identr = ident[:].bitcast(F32R)
            wk = cpool.tile([128, NKT * 256], F16, name="wk")
            nc.sync.dma_start(
                wk.rearrange("p (kt n) -> p kt n", n=256),
                wk16d.rearrange("(kt p) n -> p kt n", p=128),
            )
            wb = cpool.tile([128, NKT * H], F16, name="wb")
            nc.sync.dma_start(
                wb.rearrange("p (kt n) -> p kt n", n=H),
                wb16d.rearrange("(kt p) n -> p kt n", p=128),
            )
            bkey = cpool.tile([128, H], F32, name="bkey")
            nc.sync.dma_start(bkey[:], bkeyd[:, :])
            bbeta2 = cpool.tile([128, H], F32, name="bbeta2")
            nc.sync.dma_start(bbeta2[:], bbetad[:, :])
            onesc = cpool.tile([128, 1], F16, name="onesc")
            nc.sync.dma_start(onesc[:], onescd[:, :])
            onesr = cpool.tile([128, 128], F16, name="onesr")
            nc.sync.dma_start(onesr[:], onesrd[:, :])
            onesr16 = onesr
            bk2w = cpool.tile([1, 256], F16, name="bk2w")
            nc.sync.dma_start(bk2w[:], bk2wd[:, :])
            bb16 = cpool.tile([1, H], F16, name="bb16")
            nc.sync.dma_start(bb16[:], bb16d[:, :])

            # hiddenT fp16: [128 (d within ktile), kt*nb + b]
            hT = cpool.tile([128, NKT * nb], F16, name="hT")
            # kT'': [128 (w, two halves), (pair, g, h)] fp16, zero padded
            kT = cpool.tile([128, half * 2 * H], F16, name="kT")
            kT3 = kT.rearrange("p (j s) -> p j s", s=2 * H)
            nc.gpsimd.memset(kT[:], 0.0)
            # staging for batched softplus / rsqrt(u2): b-tile bt lives on
            # partition 32*(bt%4), column block (bt//4)*512
            ncol = ((nbt + 3) // 4) * 512
            u2all = cpool.tile([128, ncol], F32, name="u2all")
            bpall = cpool.tile([128, ncol], F32, name="bpall")
            nc.vector.memset(u2all[:], 1.0)
            nc.vector.memset(bpall[:], 0.0)

            # ---- stage A+B: hidden transpose, keys, beta, g ----
            for bt in range(nbt):
                g = 0 if bt < nbt // 2 else 1
                pbase = 64 * g
                j0 = (bt % (nbt // 2)) * 128

                hin = pa.tile([128, D], F32, name="hin", tag="hin")
                nc.sync.dma_start(hin[:], hid[bt * 128 : (bt + 1) * 128, :])
                psA = pp.tile([128, 512], F32, name="psA", tag="aux", bufs=2)
                for kt in range(NKT):
                    nc.tensor.transpose(
                        psA[:, kt * 128 : (kt + 1) * 128],
                        hin[:, kt * 128 : (kt + 1) * 128],
                        ident[:],
                    )
                hT4 = hT.rearrange("p (kt b) -> p kt b", b=nb)
                nc.vector.tensor_copy(
                    hT4[:, :, bt * 128 : (bt + 1) * 128], psA[:, 0:512]
                )

                if srank < 1:
                    continue
                # keys: per head, accumulate over k-tiles -> [64(w), 128(b)]
                # tanh(y) = 1 - 2/(exp(2y) + 1), staying in the exp/ln LUT set
                # bias folded in as a K=1 rank-1 matmul (2*b_key outer ones)
                ktmp = pb.tile([128, 512], F32, name="ktmp", tag="ktmp")
                kps = pp.tile([128, 512], F32, name="kps", tag="aux", bufs=2)
                for h in range(H):
                    for kt in range(NKT):
                        nc.tensor.matmul(
                            kps[pbase : pbase + 64, h * 128 : (h + 1) * 128],
                            wk[:, kt * 256 + h * 64 : kt * 256 + (h + 1) * 64],
                            hT[:, kt * nb + bt * 128 : kt * nb + (bt + 1) * 128],
                            start=(kt == 0),
                            stop=False,
                        )
                    nc.tensor.matmul(
                        kps[pbase : pbase + 64, h * 128 : (h + 1) * 128],
                        bk2w[0:1, h * 64 : (h + 1) * 64],
                        onesr16[0:1, 0:128],
                        start=False,
                        stop=True,
                        tile_position=(0, pbase),
                    )
                nc.scalar.activation(
                    ktmp[pbase : pbase + 64, :],
                    kps[pbase : pbase + 64, 0:512],
                    AF.Exp,
                    scale=2.0,
                )
                khalf = ktmp[pbase : pbase + 64, :]
                nc.vector.tensor_scalar_add(khalf, khalf, 1.0)
                nc.vector.reciprocal(khalf, khalf)
                slots_all = kT3[pbase : pbase + 64, j0 : j0 + 128, g * H : g * H + H]
                nc.vector.tensor_scalar(
                    slots_all,
                    ktmp.rearrange("p (h j) -> p j h", h=H)[pbase : pbase + 64, :, :],
                    -2.0,
                    1.0,
                    ALU.mult,
                    ALU.add,
                )

                # u2 = sum_w key^2 (unscaled): square then ones-matmul
                ksq = pb.tile([128, 512], F16, name="ksq", tag="ksq")
                ksq3 = ksq.rearrange("p (j h) -> p j h", h=H)
                slots = kT3[pbase : pbase + 64, j0 : j0 + 128, g * H : g * H + H]
                nc.vector.tensor_mul(ksq3[pbase : pbase + 64, :, :], slots, slots)
                u2ps = pp.tile([128, 512], F32, name="u2ps", tag="numT", bufs=2)
                pk0 = 32 * (bt % 4)
                nc.tensor.matmul(
                    u2ps[pk0 : pk0 + 1, :],
                    onesc[pbase : pbase + 64, :],
                    ksq[pbase : pbase + 64, :],
                    start=True,
                    stop=True,
                    tile_position=(pbase, pk0),
                )

                # u2 + EPS into the staging tile (partition pk, col block ck)
                pk = pk0
                ck = (bt // 4) * 512
                nc.vector.tensor_scalar_add(
                    u2all[pk : pk + 1, ck : ck + 512], u2ps[pk : pk + 1, :], EPS
                )

                # beta (pre-softplus, incl. bias): [1, (h, b)]
                bps = pp.tile([128, 512], F32, name="bps", tag="aux", bufs=2)
                for h in range(H):
                    for kt in range(NKT):
                        nc.tensor.matmul(
                            bps[pk : pk + 1, h * 128 : (h + 1) * 128],
                            wb[:, kt * H + h : kt * H + h + 1],
                            hT[:, kt * nb + bt * 128 : kt * nb + (bt + 1) * 128],
                            start=(kt == 0),
                            stop=False,
                            tile_position=(0, pk),
                        )
                    nc.tensor.matmul(
                        bps[pk : pk + 1, h * 128 : (h + 1) * 128],
                        bb16[0:1, h : h + 1],
                        onesr16[0:1, 0:128],
                        start=False,
                        stop=True,
                        tile_position=(0, pk),
                    )
                nc.vector.tensor_copy(
                    bpall[pk : pk + 1, ck : ck + 512], bps[pk : pk + 1, 0:512]
                )

            # ---- batched softplus + rsqrt(u2), all within the exp/ln LUT set ----
            # softplus(x) = ln(1 + exp(x)); x in ~[-3.5, 3.5] so exp is safe.
            sp1 = pb.tile([128, ncol], F32, name="sp1", tag="sp1", bufs=1)
            if srank >= 1:
                nc.scalar.activation(sp1[:], bpall[:], AF.Exp)
                nc.vector.tensor_scalar_add(sp1[:], sp1[:], 1.0)
                nc.scalar.activation(sp1[:], sp1[:], AF.Ln)  # = softplus, (bt, h, j)
            # rsqrt(x) = exp(-0.5 * ln(x))
            ru2 = pb.tile([128, ncol], F32, name="ru2", tag="ru2", bufs=1)
            gall = pb.tile([128, ncol], F16, name="gall", tag="gall", bufs=1)
            if srank >= 1:
                nc.scalar.activation(ru2[:], u2all[:], AF.Ln)
                nc.scalar.activation(ru2[:], ru2[:], AF.Exp, scale=-0.5)
                g4 = gall.rearrange("p (c j h) -> p c j h", h=H, j=128)
                su4 = ru2.rearrange("p (c j h) -> p c j h", h=H, j=128)
                bt4 = sp1.rearrange("p (c h j) -> p c j h", h=H, j=128)
                nc.vector.tensor_mul(g4[:, :, :, :], su4[:, :, :, :], bt4[:, :, :, :])

            # fold g into kT'' per b-tile
            for bt in range(nbt if srank >= 1 else 0):
                g = 0 if bt < nbt // 2 else 1
                pbase = 64 * g
                j0 = (bt % (nbt // 2)) * 128
                pk = 32 * (bt % 4)
                ck = (bt // 4) * 512
                slots = kT3[pbase : pbase + 64, j0 : j0 + 128, g * H : g * H + H]
                gB = pp.tile([128, 512], F32, name="gB", tag="aux", bufs=2)
                nc.tensor.matmul(
                    gB[pbase : pbase + 64, :],
                    onesr[pk : pk + 1, 0:64],
                    gall[pk : pk + 1, ck : ck + 512],
                    start=True,
                    stop=True,
                    tile_position=(pk, pbase),
                )
                gB3 = gB.rearrange("p (j h) -> p j h", h=H)
                nc.vector.tensor_mul(slots, slots, gB3[pbase : pbase + 64, :, :])

            # ---- stage C: memory pipeline ----
            # last two groups are half-size to shorten the data-dependent
            # pipeline tail after the final memory DMA lands
            if ngrp >= 2:
                blocks = [(g * GROUP, GROUP) for g in range(ngrp - 1)]
                blocks += [((ngrp - 1) * GROUP, GROUP // 2),
                           ((ngrp - 1) * GROUP + GROUP // 2, GROUP // 2)]
            else:
                blocks = [(g * GROUP, GROUP) for g in range(ngrp)]
            for b0, gsz in (blocks if srank >= 2 else []):
                mst = pc.tile([128, gsz * 128], F32, name="mst", tag="mst", bufs=2)
                mst4 = mst.rearrange("p (i g w) -> p i g w", g=2, w=W)
                nc.sync.dma_start(
                    mst4[:, :, 0, :], mem[b0 : b0 + gsz, :, :].rearrange("i m w -> m i w")
                )
                nc.sync.dma_start(
                    mst4[:, :, 1, :],
                    mem[half + b0 : half + b0 + gsz, :, :].rearrange("i m w -> m i w"),
                )

                # v2 = sum_w mem^2: ACT squares + fp16 pairwise-add tree on
                # DVE (fp16 tensor_tensor runs in the 2x DVE mode)
                msq = pc.tile([128, gsz * 128], F16, name="msq", tag="msq", bufs=1)
                v2 = pc.tile([128, 2 * gsz], F32, name="v2", tag="v2", bufs=2)
                qw = gsz * 32
                for q in range(4):
                    nc.scalar.activation(
                        msq[:, q * qw : (q + 1) * qw],
                        mst[:, q * qw : (q + 1) * qw],
                        AF.Square,
                    )
                ta = pc.tile([128, gsz * 64], F16, name="ta", tag="ta", bufs=1)
                tb = pc.tile([128, gsz * 32], F16, name="tb", tag="tb", bufs=1)
                msqv = msq.rearrange("p (c w) -> p c w", w=W)
                nc.vector.tensor_add(
                    ta.rearrange("p (c w) -> p c w", w=32)[:, :, :],
                    msqv[:, :, 0:32],
                    msqv[:, :, 32:64],
                )
                tav = ta.rearrange("p (c w) -> p c w", w=32)
                nc.vector.tensor_add(
                    tb.rearrange("p (c w) -> p c w", w=16)[:, :, :],
                    tav[:, :, 0:16],
                    tav[:, :, 16:32],
                )
                tbv = tb.rearrange("p (c w) -> p c w", w=16)
                ta2 = ta[:, 0 : gsz * 16].rearrange("p (c w) -> p c w", w=8)
                nc.vector.tensor_add(ta2[:, :, :], tbv[:, :, 0:8], tbv[:, :, 8:16])
                tb2 = tb[:, 0 : gsz * 8].rearrange("p (c w) -> p c w", w=4)
                nc.vector.tensor_add(tb2[:, :, :], ta2[:, :, 0:4], ta2[:, :, 4:8])
                ta3 = ta[:, 0 : gsz * 4].rearrange("p (c w) -> p c w", w=2)
                nc.vector.tensor_add(ta3[:, :, :], tb2[:, :, 0:2], tb2[:, :, 2:4])
                # final level in fp32, fold in EPS
                nc.vector.tensor_add(
                    v2.rearrange("p (c w) -> p c w", w=1)[:, :, :],
                    ta3[:, :, 0:1],
                    ta3[:, :, 1:2],
                )
                nc.vector.tensor_scalar_add(v2[:], v2[:], EPS)
                rsq = pc.tile([128, 2 * gsz], F32, name="rsq", tag="rsq", bufs=2)
                nc.scalar.activation(rsq[:], v2[:], AF.Ln)
                nc.scalar.activation(rsq[:], rsq[:], AF.Exp, scale=-0.5)

                # transpose mem pairs; copy-cast PSUM -> fp16 SBUF
                memT = pc.tile([128, gsz * 128], F16, name="memT", tag="memT", bufs=2)
                for i in range(gsz if srank >= 3 else 0):
                    if i % 8 == 0:
                        tps = pp.tile([128, 1024], F32, name="tps", tag="tps", bufs=2)
                    nc.tensor.transpose(
                        tps[:, (i % 8) * 128 : (i % 8 + 1) * 128],
                        mst[:, i * 128 : (i + 1) * 128],
                        ident[:],
                    )
                    if i % 8 == 7:
                        q = i // 8
                        dst = memT[:, q * 1024 : (q + 1) * 1024]
                        if q % 2 == 0:
                            nc.scalar.activation(dst, tps[:], AF.Copy)
                        else:
                            nc.vector.tensor_copy(dst, tps[:])

                # einsum: one matmul per pair
                numT = pp.tile([128, 512], F32, name="numT", tag="numT", bufs=2)
                for i in range(gsz if srank >= 4 else 0):
                    nc.tensor.matmul(
                        numT[:, i * 8 : (i + 1) * 8],
                        memT[:, i * 128 : (i + 1) * 128],
                        kT[:, (b0 + i) * 8 : (b0 + i + 1) * 8],
                        start=True,
                        stop=True,
                    )

                if srank >= 5:
                    # logits = numT * rsqrt(v2); exp; sum over m; normalize
                    nw = gsz * 8
                    logit = pc.tile([128, nw], F32, name="logit", tag="logit", bufs=2)
                    logit4 = logit.rearrange("p (i g h) -> p i g h", g=2, h=H)
                    numT4 = numT[:, 0:nw].rearrange("p (i g h) -> p i g h", g=2, h=H)
                    rb = (
                        rsq.rearrange("p (i g) -> p i g", g=2)
                        .unsqueeze(3)
                        .to_broadcast([128, gsz, 2, H])
                    )
                    nc.vector.tensor_mul(logit4[:, :, :, :], numT4[:, :, :, :], rb)
                    expT = pc.tile([128, nw], F16, name="expT", tag="expT", bufs=2)
                    nc.scalar.activation(expT[:], logit[:], AF.Exp)
                    sums = pp.tile([128, 512], F32, name="sums", tag="aux", bufs=2)
                    nc.tensor.matmul(
                        sums[0:1, 0:nw], onesc[:, :], expT[:], start=True, stop=True
                    )
                    rc32 = pc.tile([1, nw], F32, name="rc32", tag="rc32", bufs=2)
                    nc.vector.reciprocal(rc32[:], sums[0:1, 0:nw])
                    rc16 = pc.tile([1, nw], F16, name="rc16", tag="rc16", bufs=2)
                    nc.vector.tensor_copy(rc16[:], rc32[:])
                    rB = pp.tile([128, 512], F32, name="rB", tag="aux", bufs=2)
                    nc.tensor.matmul(
                        rB[:, 0:nw], onesr[0:1, :], rc16[:], start=True, stop=True
                    )
                    # normalize; write columns reordered to (g, i, h) so each
                    # transpose-back quarter covers one batch-half contiguously
                    wcsb = pc.tile([128, nw], F32, name="wcsb", tag="wcsb", bufs=2)
                    wcsb_v = wcsb.rearrange("p (g i h) -> p i g h", g=2, h=H)
                    expT4 = expT.rearrange("p (i g h) -> p i g h", g=2, h=H)
                    rB4 = rB[:, 0:nw].rearrange("p (i g h) -> p i g h", g=2, h=H)
                    nc.vector.tensor_mul(
                        wcsb_v[:, :, :, :], expT4[:, :, :, :], rB4[:, :, :, :]
                    )

                    # transpose back to [b, h, m] and store
                    wcr = wc.rearrange("(g bb) h m -> bb g h m", g=2)
                    nhalf = gsz // 32
                    for q in range(nw // 128):
                        gg = q // nhalf
                        i0 = b0 + (q % nhalf) * 32
                        wcT = pp.tile([128, 512], F32, name="wcT", tag="numT", bufs=2)
                        nc.tensor.transpose(
                            wcT[:, 0:128], wcsb[:, q * 128 : (q + 1) * 128], ident[:]
                        )
                        wcO = pc.tile([128, 128], F32, name="wcO", tag="wcO", bufs=2)
                        nc.vector.tensor_copy(wcO[:], wcT[:, 0:128])
                        nc.sync.dma_start(wcr[i0 : i0 + 32, gg, :, :], wcO[:])

            if srank < 5:
                zout = cpool.tile([128, H * M], F32, name="zout")
                nc.vector.memset(zout[:], 0.0)
                for t in range(nb // 128):
                    nc.sync.dma_start(
                        wc[t * 128 : (t + 1) * 128, :, :].rearrange("b h m -> b (h m)"),
                        zout[:],
                    )

    nc.compile()
    return nc


_CACHE = {}


def _get_nc(nb=NB):
    if nb not in _CACHE:
        _CACHE[nb] = build_bass(nb)
    return _CACHE[nb]


def _make_consts(W_key, b_key, W_beta, b_beta):
    # bkey is pre-doubled: the tanh synthesis computes exp(2*y + bkey)
    bk = 2.0 * np.asarray(b_key, np.float32).reshape(H, W).T  # [64, 4]
    return {
        "wk16": np.asarray(W_key).astype(np.float16),
        "wb16": np.asarray(W_beta).astype(np.float16),
        "bkey": np.concatenate([bk, bk], axis=0).astype(np.float32),
        "bbeta": np.tile(np.asarray(b_beta, np.float32).reshape(1, H), (128, 1)),
        "ident": np.eye(128, dtype=np.float32),
        "onesc": np.ones([128, 1], np.float16),
        "onesr": np.ones([128, 128], np.float16),
        "bk2w": np.asarray(b_key, np.float32).reshape(1, 256).astype(np.float16),
        "bb16": np.asarray(b_beta, np.float32).reshape(1, H).astype(np.float16),
    }


def kernel(hidden_vb, memory_vb, W_key, b_key, W_beta, b_beta):
    hidden_vb = np.ascontiguousarray(np.asarray(hidden_vb, np.float32))
    memory_vb = np.ascontiguousarray(np.asarray(memory_vb, np.float32))
    B = hidden_vb.shape[0]
    nb = B // NCORES
    nc = _get_nc(nb)
    consts = _make_consts(W_key, b_key, W_beta, b_beta)
    in_maps = []
    for c in range(NCORES):
        m = dict(consts)
        m["hidden"] = hidden_vb[c * nb : (c + 1) * nb]
        m["mem"] = memory_vb[c * nb : (c + 1) * nb]
        in_maps.append(m)
    res = bass_utils.run_bass_kernel_spmd(nc, in_maps, core_ids=list(range(NCORES)))
    return np.concatenate([res.results[c]["wc"] for c in range(NCORES)], axis=0)

